# revision 2
# baseline (speedup 1.0000x reference)
"""Trainium2 Bass kernel for nn_ChimeraNet (encoder -> 10-step Euler RNN -> LN -> readout).

Data-parallel over 8 NeuronCores: each core gets 1024 rows of the batch and a
replicated set of (host-prefolded) weights.

Math (per core, R=1024 rows, D=1024), in "drive space" z = h @ W_res + c:
    c   = x @ W_c + bias               with W_c = W_enc.T @ W_in (host-folded)
    z_0 = c;  T_k = tanh(z_k)
    z_{k+1} = 0.8 z_k + 0.2 c + 0.2 (T_k @ W_res)      k = 0..8
    u_{k+1} = 0.8 u_k + T_k                            k = 0..9,  u_0 = 0
    h = 0.2 u_10;  out = LayerNorm(h) @ W_out.T + b_out (folded like before)

The z state is stored in the exponentially rescaled frame G_k = z_k / 0.8^k so
each step's state update is a single one-scalar DVE op:
    G_{k+1} = G_k + q_k * psum,   q_k = 0.2/(16*0.8^{k+1})
    psum    = 16*c (identity matmul, bf16) + T8 @ fp8(16*W_res)  (DoubleRow fp8)
    T_k     = tanh(0.8^k * G_k)   (ACT with scale, bf16 out)

fp8 e4m3 is used ONLY for the recurrent matmul operands (T8 = fp8 cast of the
bf16 tanh, W8 = fp8(16*W_res)); the u accumulator consumes the exact bf16 tanh,
which keeps the final relative error ~7e-3 (vs ~1.6e-2 if u saw fp8).
DoubleRow mode contracts 2 k-subtiles per matmul at 0.5 cyc/row -> ~2x PE.

On-chip layout: transposed state (D on partitions, rows on free dim), matmuls
weight-stationary. LayerNorm stats via ones-column readout + squared-tile
matmuls, LN folded into the readout (same tail as before, bf16 operands with a
bf16 residual split of the readout weights for accuracy).
"""

import os
import sys

import numpy as np

try:
    import concourse.bass as bass  # noqa: F401
except ImportError:  # pragma: no cover - fresh grading env without PYTHONPATH
    for p in ("/root/.axon_site", "/root/.axon_site/_ro/trn_rl_repo",
              "/root/.axon_site/_ro/pypackages", "/opt/trn_rl_repo"):
        if os.path.isdir(p) and p not in sys.path:
            sys.path.append(p)
    import concourse.bass as bass

from contextlib import ExitStack

import ml_dtypes
import concourse.tile as tile
from concourse import bacc, bass_utils, mybir
from concourse.masks import make_identity

N_CORES = 8
B = 8192
R = B // N_CORES        # rows per core
D = 1024                # latent dim
KX = 784                # encoder input dim
DT_STEP = 0.2
STEPS = 10
EPS = 1e-5
SW = 16.0               # fp8 weight upscale (exact in bf16)

F32 = mybir.dt.float32
F32R = mybir.dt.float32r
BF16 = mybir.dt.bfloat16
F8 = mybir.dt.float8e4
AF = mybir.ActivationFunctionType
ALU = mybir.AluOpType
DR = mybir.MatmulPerfMode.DoubleRow

KD = D // 128           # 8 k/m tiles over D
NS = R // 512           # 2 moving-dim slices of 512
KXT = [128] * 6 + [16]  # 784 = 6*128 + 16
NWARM = 6               # PE warmup matmuls (HAM un-throttle during DMA wait)

CAST_ON_GPSIMD = os.environ.get("CAST_ENGINE", "gpsimd") == "gpsimd"


def _build_program():
    nc = bacc.Bacc("TRN2", target_bir_lowering=False, debug=False)

    x = nc.dram_tensor("x", [R, KX], F32, kind="ExternalInput").ap()
    w_c = nc.dram_tensor("w_c", [KX, D], F32, kind="ExternalInput").ap()
    w8 = nc.dram_tensor("w8", [128, KD, D], F8, kind="ExternalInput").ap()
    bias = nc.dram_tensor("bias", [D], F32, kind="ExternalInput").ap()
    w2a = nc.dram_tensor("w2a", [128, KD, 11], BF16, kind="ExternalInput").ap()
    w2r = nc.dram_tensor("w2r", [128, KD, 11], BF16, kind="ExternalInput").ap()
    w1 = nc.dram_tensor("w1", [10], F32, kind="ExternalInput").ap()
    b2 = nc.dram_tensor("b2", [10], F32, kind="ExternalInput").ap()
    out = nc.dram_tensor("out", [R, 10], F32, kind="ExternalOutput").ap()

    cast_eng = None  # set inside

    with tile.TileContext(nc) as tc, ExitStack() as ctx:
        state = ctx.enter_context(tc.tile_pool(name="state", bufs=1))
        consts = ctx.enter_context(tc.tile_pool(name="consts", bufs=1))
        wres_pool = ctx.enter_context(tc.tile_pool(name="wres", bufs=1))

        # persistent SBUF state
        g = [[state.tile([128, R], BF16, name=f"g{b}_{k}", tag=f"g{b}_{k}") for k in range(KD)]
             for b in range(2)]
        u = [state.tile([128, R], BF16, name=f"u{k}", tag=f"u{k}") for k in range(KD)]
        drive = [state.tile([128, R], BF16, name=f"dr{k}", tag=f"dr{k}") for k in range(KD)]
        t8 = [state.tile([128, KD, 512], F8, name=f"t8_{n}", tag=f"t8_{n}") for n in range(NS)]
        w8_sb = wres_pool.tile([128, KD, D], F8, name="w8", tag="w8")

        with ExitStack() as mmctx:
            psum = mmctx.enter_context(
                tc.tile_pool(name="mm", bufs=4, space="PSUM"))
            if True:
                # PE warmup: dependency-free fp32 matmuls starting at t~0 pull
                # the HAM clock gate to 8/8 while the input DMAs are in flight.
                warm_src = consts.tile([128, 256], F32)
                nc.vector.memset(warm_src, 0.01)
                warm_sb = consts.tile([128, 1], F32)
                for w in range(NWARM):
                    wp = psum.tile([128, 512], F32, name=f"warm{w}", tag="mm")
                    nc.tensor.matmul(wp[:, :256], lhsT=warm_src[:, :128], rhs=warm_src,
                                     start=True, stop=True)
                    if w == NWARM - 1:
                        nc.vector.tensor_copy(warm_sb, wp[:, :1])  # keep-alive

                ident = consts.tile([128, 128], F32)
                make_identity(nc, ident)
                identB = consts.tile([128, 128], BF16)
                nc.scalar.mul(identB, ident, SW)   # 16*I for the drive matmul
                bias_sb = consts.tile([128, KD], F32)
                nc.gpsimd.dma_start(out=bias_sb, in_=bias.rearrange("(m p) -> p m", p=128))

                # ------------ encoder: x -> x.T, c = x @ W_c + bias (bf16) ----
                with ExitStack() as enc:
                    xn_pool = enc.enter_context(tc.tile_pool(name="xn", bufs=4))
                    xt_pool = enc.enter_context(tc.tile_pool(name="xt", bufs=1))
                    wc_pool = enc.enter_context(tc.tile_pool(name="wc", bufs=1))
                    etp = enc.enter_context(
                        tc.tile_pool(name="etp", bufs=4, space="PSUM"))

                    xt_big = xt_pool.tile([128, len(KXT), R], F32R, name="xt_big")
                    wc_sb = [wc_pool.tile([128, D], F32R, name=f"wc{k}", tag=f"wc{k}")
                             for k in range(len(KXT))]
                    for k, kw in enumerate(KXT):
                        nc.scalar.dma_start(out=wc_sb[k][:kw, :],
                                            in_=w_c[k * 128:k * 128 + kw, :].bitcast(F32R))

                    def transpose_rt(rt):
                        xn = xn_pool.tile([128, KX], F32, name=f"xn{rt}", tag="xn")
                        nc.sync.dma_start(out=xn, in_=x[rt * 128:(rt + 1) * 128, :])
                        rsl = slice(rt * 128, (rt + 1) * 128)
                        wp0 = psum.tile([128, 512], F32, name=f"wmh{rt}", tag="mm")
                        nc.tensor.matmul(wp0[:, :256], lhsT=warm_src[:, :128],
                                         rhs=warm_src, start=True, stop=True)
                        for kp in range(3):
                            pt = etp.tile([128, 256], F32, name=f"pt{rt}_{kp}", tag="tp")
                            for h in range(2):
                                k = 2 * kp + h
                                nc.tensor.transpose(pt[:, h * 128:(h + 1) * 128],
                                                    xn[:, k * 128:(k + 1) * 128], ident)
                            src = pt.rearrange("p (two c) -> p two c", two=2)
                            dst = xt_big[:, 2 * kp:2 * kp + 2, rsl]
                            if kp % 2 == 0:
                                nc.scalar.copy(dst, src)
                            else:
                                nc.vector.tensor_copy(dst, src)
                        pt = etp.tile([128, 256], F32, name=f"pt{rt}_3", tag="tp")
                        nc.tensor.transpose(pt[:16, :128], xn[:, 768:784], ident)
                        nc.vector.tensor_copy(xt_big[:16, 6, rsl], pt[:16, :128])

                    def encoder_mms(n):
                        sl = slice(n * 512, (n + 1) * 512)
                        for m in range(KD):
                            ps = psum.tile([128, 512], F32, name=f"eps{n}_{m}", tag="mm")
                            for k, kw in enumerate(KXT):
                                nc.tensor.matmul(
                                    ps,
                                    lhsT=wc_sb[k][:kw, m * 128:(m + 1) * 128],
                                    rhs=xt_big[:kw, k, sl],
                                    start=(k == 0), stop=(k == len(KXT) - 1))
                            nc.scalar.activation(drive[m][:, sl], ps, AF.Identity,
                                                 bias=bias_sb[:, m:m + 1], scale=1.0)

                    for rt in range(4):
                        transpose_rt(rt)
                    encoder_mms(0)
                    for rt in range(4, 8):
                        transpose_rt(rt)
                    encoder_mms(1)

                # W8 arrives on the gpsimd queue while the encoder runs.
                nc.gpsimd.dma_start(out=w8_sb, in_=w8)

                # ------------ Euler integration loop (G-frame) ----------------
                tau_pool = ctx.enter_context(tc.tile_pool(name="tau", bufs=6))
                sqp = ctx.enter_context(tc.tile_pool(name="sq", bufs=1))
                sq_tiles = [[sqp.tile([128, 512], BF16, name=f"sq{n}_{k}", tag=f"sq{n}_{k}")
                             for k in range(KD)] for n in range(NS)]

                cast_eng = nc.gpsimd if CAST_ON_GPSIMD else nc.vector

                for s in range(STEPS):
                    ak = float(0.8 ** s)
                    qk = float(DT_STEP / (SW * 0.8 ** (s + 1)))
                    cur = drive if s == 0 else g[s % 2]
                    nxt = g[(s + 1) % 2]
                    # tanh + fp8 cast + u update, per (n, m) tile
                    for n in range(NS):
                        sl = slice(n * 512, (n + 1) * 512)
                        for m in range(KD):
                            tau = tau_pool.tile([128, 512], BF16,
                                                name=f"tau{s}_{n}_{m}", tag="tau")
                            nc.scalar.activation(tau, cur[m][:, sl], AF.Tanh, scale=ak)
                            cast_eng.tensor_copy(t8[n][:, m, :], tau)
                            if s == 0:
                                nc.vector.tensor_copy(u[m][:, sl], tau)
                            else:
                                nc.vector.scalar_tensor_tensor(
                                    u[m][:, sl], in0=u[m][:, sl], scalar=1.0 - DT_STEP,
                                    in1=tau, op0=ALU.mult, op1=ALU.add)
                            if s == STEPS - 1:
                                nc.vector.tensor_mul(sq_tiles[n][m], u[m][:, sl],
                                                     u[m][:, sl])
                    if s == STEPS - 1:
                        break
                    # PE: psum = 16*c + T8 @ W8, both slices interleaved so the
                    # DoubleRow stationaries are shared between the two slices.
                    for m in range(KD):
                        ps = [psum.tile([128, 512], F32, name=f"ps{s}_{n}_{m}", tag="mm")
                              for n in range(NS)]
                        for n in range(NS):
                            nc.tensor.matmul(ps[n], lhsT=identB,
                                             rhs=drive[m][:, n * 512:(n + 1) * 512],
                                             start=True, stop=False)
                        for j in range(KD // 2):
                            lhsT = w8_sb[:, 2 * j:2 * j + 2, m * 128:(m + 1) * 128]
                            for n in range(NS):
                                nc.tensor.matmul(ps[n], lhsT=lhsT,
                                                 rhs=t8[n][:, 2 * j:2 * j + 2, :],
                                                 perf_mode=DR,
                                                 start=False, stop=(j == KD // 2 - 1))
                        for n in range(NS):
                            sl = slice(n * 512, (n + 1) * 512)
                            nc.vector.scalar_tensor_tensor(
                                nxt[m][:, sl], in0=ps[n], scalar=qk,
                                in1=cur[m][:, sl], op0=ALU.mult, op1=ALU.add)

                gfin = u

                # ------------ tail: LN stats + readout (matmul part) ----------
                tail = ctx.enter_context(tc.tile_pool(name="tail", bufs=1))

                ones_sb = tail.tile([128, 1], BF16)
                nc.vector.memset(ones_sb, 1.0)
                eps_sb = tail.tile([128, 1], F32)
                nc.vector.memset(eps_sb, EPS)
                # w2a/w2r = bf16 hi/lo split of [0.2*W2.T | ones]
                w2a_sb = tail.tile([128, KD, 11], BF16)
                nc.gpsimd.dma_start(out=w2a_sb, in_=w2a)
                w2r_sb = tail.tile([128, KD, 11], BF16)
                nc.gpsimd.dma_start(out=w2r_sb, in_=w2r)
                w1_bc = tail.tile([128, 10], F32)
                nc.gpsimd.dma_start(out=w1_bc, in_=bass.AP(tensor=w1.tensor, offset=w1.offset,
                                                           ap=[[0, 128]] + list(w1.ap)))
                b2_bc = tail.tile([128, 10], F32)
                nc.gpsimd.dma_start(out=b2_bc, in_=bass.AP(tensor=b2.tensor, offset=b2.offset,
                                                           ap=[[0, 128]] + list(b2.ap)))

                s2_sb = tail.tile([1, R], F32)
                y_sb = tail.tile([11, R], F32)

                tp2ctx = ExitStack()
                tp2 = tp2ctx.enter_context(
                    tc.tile_pool(name="tp2", bufs=4, space="PSUM"))
                for n in range(NS):
                    sl = slice(n * 512, (n + 1) * 512)
                    yp = psum.tile([11, 512], F32, name=f"yp{n}", tag="mm")
                    for k in range(KD):
                        nc.tensor.matmul(yp, lhsT=w2a_sb[:, k, :],
                                         rhs=gfin[k][:, sl],
                                         start=(k == 0), stop=False)
                    for k in range(KD):
                        nc.tensor.matmul(yp, lhsT=w2r_sb[:, k, :],
                                         rhs=gfin[k][:, sl],
                                         start=False, stop=(k == KD - 1))
                    nc.scalar.copy(y_sb[:, sl], yp)
                    s2 = psum.tile([1, 512], F32, name=f"s2p{n}", tag="mm")
                    for k in range(KD):
                        nc.tensor.matmul(s2, lhsT=ones_sb, rhs=sq_tiles[n][k],
                                         start=(k == 0), stop=(k == KD - 1))
                    nc.scalar.copy(s2_sb[:, sl], s2)

                    for rt in range(n * 4, (n + 1) * 4):
                        sl = slice(rt * 128, (rt + 1) * 128)
                        yn = tp2.tile([128, 11], F32, name=f"yn{rt}", tag="st")
                        nc.tensor.transpose(yn, y_sb[:, sl], ident[:11, :11])
                        p2 = tp2.tile([128, 1], F32, name=f"p2_{rt}", tag="st")
                        nc.tensor.transpose(p2, s2_sb[:, sl], ident[:1, :1])
                        mu_n = tail.tile([128, 1], F32, name=f"mu{rt}", tag="mu", bufs=2)
                        nc.scalar.mul(mu_n, yn[:, 10:11], -DT_STEP / D)   # -mean(h)
                        ex2 = tail.tile([128, 1], F32, name=f"ex2_{rt}", tag="ex2", bufs=2)
                        nc.scalar.mul(ex2, p2, DT_STEP * DT_STEP / D)     # E[h^2]
                        var = tail.tile([128, 1], F32, name=f"var{rt}", tag="var", bufs=2)
                        nc.vector.scalar_tensor_tensor(var, in0=mu_n, scalar=-1.0,
                                                       op0=ALU.mult, in1=mu_n,
                                                       op1=ALU.mult)
                        nc.vector.tensor_add(var, var, ex2)
                        sd = tail.tile([128, 1], F32, name=f"sd{rt}", tag="sd", bufs=2)
                        nc.scalar.activation(sd, var, AF.Sqrt, bias=eps_sb, scale=1.0)
                        inv = tail.tile([128, 1], F32, name=f"inv{rt}", tag="inv", bufs=2)
                        nc.vector.reciprocal(inv, sd)
                        qn = tail.tile([128, 1], F32, name=f"qn{rt}", tag="qn", bufs=2)
                        nc.vector.tensor_mul(qn, mu_n, inv)               # -mu*inv

                        t1 = tail.tile([128, 10], F32, name=f"t1_{rt}", tag="t1", bufs=2)
                        nc.vector.tensor_scalar_mul(t1, yn[:, 0:10], inv)
                        t2 = tail.tile([128, 10], F32, name=f"t2_{rt}", tag="t2", bufs=2)
                        nc.vector.scalar_tensor_tensor(t2, in0=w1_bc, scalar=qn,
                                                       in1=t1, op0=ALU.mult, op1=ALU.add)
                        o = tail.tile([128, 10], F32, name=f"o{rt}", tag="o", bufs=2)
                        nc.vector.tensor_add(o, t2, b2_bc)
                        nc.sync.dma_start(out=out[sl, :], in_=o)
                tp2ctx.close()

    nc.compile()
    return nc


_NC_CACHE = None


def _get_program():
    global _NC_CACHE
    if _NC_CACHE is None:
        _NC_CACHE = _build_program()
    return _NC_CACHE


def _prepare_in_maps(inputs):
    x = np.asarray(inputs["x"], dtype=np.float32)
    w_enc = np.asarray(inputs["W_enc"], dtype=np.float32)
    w_res = np.asarray(inputs["W_res"], dtype=np.float32)
    w_in = np.asarray(inputs["W_in"], dtype=np.float32)
    bias = np.asarray(inputs["bias"], dtype=np.float32)
    ln_g = np.asarray(inputs["ln_g"], dtype=np.float32)
    ln_b = np.asarray(inputs["ln_b"], dtype=np.float32)
    w_out = np.asarray(inputs["W_out"], dtype=np.float32)
    b_out = np.asarray(inputs["b_out"], dtype=np.float32)

    w_c = (w_enc.T.astype(np.float64) @ w_in.astype(np.float64)).astype(np.float32)
    w2 = w_out * ln_g[None, :]                       # [10, D]

    # fp8 recurrent weights, upscaled by SW, layout [p, ksub, m]
    w8 = (SW * w_res).astype(ml_dtypes.float8_e4m3)
    w8 = np.ascontiguousarray(w8.reshape(KD, 128, D).transpose(1, 0, 2))

    # readout: [0.2*W2.T | ones] in bf16 hi + bf16 residual, layout [p, k, o]
    a = np.empty((D, 11), np.float64)
    a[:, :10] = DT_STEP * w2.T.astype(np.float64)
    a[:, 10] = 1.0
    a16 = a.astype(ml_dtypes.bfloat16)
    ar16 = (a - a16.astype(np.float64)).astype(ml_dtypes.bfloat16)
    a16 = np.ascontiguousarray(a16.reshape(KD, 128, 11).transpose(1, 0, 2))
    ar16 = np.ascontiguousarray(ar16.reshape(KD, 128, 11).transpose(1, 0, 2))

    w1v = w2.sum(axis=1).astype(np.float32)
    b2v = (w_out.astype(np.float64) @ ln_b.astype(np.float64)
           + b_out.astype(np.float64)).astype(np.float32)

    shared = {
        "w_c": np.ascontiguousarray(w_c),
        "w8": w8,
        "bias": np.ascontiguousarray(bias),
        "w2a": a16,
        "w2r": ar16,
        "w1": np.ascontiguousarray(w1v),
        "b2": np.ascontiguousarray(b2v),
    }
    in_maps = []
    for c in range(N_CORES):
        m = dict(shared)
        m["x"] = np.ascontiguousarray(x[c * R:(c + 1) * R, :])
        in_maps.append(m)
    return in_maps


def run(inputs, trace=False, tmpdir=None):
    """Run on 8 NeuronCores; returns (out [8192,10], BassKernelResults)."""
    nc = _get_program()
    in_maps = _prepare_in_maps(inputs)
    res = bass_utils.run_bass_kernel_spmd(
        nc, in_maps, core_ids=list(range(N_CORES)), trace=trace, tmpdir=tmpdir)
    outs = [np.asarray(r["out"]) for r in res.results]
    return np.concatenate(outs, axis=0), res


def kernel(**inputs):
    out, _ = run(inputs, trace=False)
    return out


# revision 10
# speedup vs baseline: 1.4752x; 1.4752x over previous
"""Trainium2 Bass kernel for nn_ChimeraNet (encoder -> 10-step Euler RNN -> LN -> readout).

Data-parallel over 8 NeuronCores: each core gets 1024 rows of the batch and a
replicated set of (host-prefolded) weights.

Math (per core, R=1024 rows, D=1024), in "drive space" z = h @ W_res + c:
    c   = x @ W_c + bias               with W_c = W_enc.T @ W_in (host-folded)
    z_0 = c;  T_k = tanh(z_k)
    z_{k+1} = 0.8 z_k + 0.2 c + 0.2 (T_k @ W_res)      k = 0..8
    u_{k+1} = 0.8 u_k + T_k                            k = 0..9,  u_0 = 0
    h = 0.2 u_10;  out = LayerNorm(h) @ W_out.T + b_out (folded like before)

The z state is kept in the exponentially rescaled+upscaled frame
G_k = 16 z_k / 0.8^k (fp32) so each step's state update is a single
one-scalar DVE op reading the matmul PSUM directly:
    G_{k+1} = G_k + 1.25^{k+1} * psum
    psum    = 16 c (f32r identity matmul) + T8 @ fp8(16 W_res)  (DoubleRow fp8)
    T_k     = tanh((0.8^k/16) * G_k)   (ACT with scale, bf16 out)
The drive tiles store 16c (scale folded into the encoder eviction), so G_0 IS
the drive tile and no init op is needed.

fp8 e4m3 is used ONLY for the recurrent matmul operands (T8 = fp8 cast of the
bf16 tanh, on ACT; W8 = fp8(16*W_res)); the u accumulator consumes the exact
bf16 tanh, which keeps the final relative error ~7e-3.
DoubleRow contracts 2 k-subtiles per matmul at 0.5 cyc/row -> ~2x PE on the
dominant recurrent matmul.

Elementwise ops run at FD=1024 (full row range) to amortize per-op overheads;
the per-m-tile PSUM is a 2-bank [128,1024] tile whose halves are filled by
N=512 matmuls (both row slices share each DoubleRow stationary back-to-back).
"""

import os
import sys

import numpy as np

try:
    import concourse.bass as bass  # noqa: F401
except ImportError:  # pragma: no cover - fresh grading env without PYTHONPATH
    for p in ("/root/.axon_site", "/root/.axon_site/_ro/trn_rl_repo",
              "/root/.axon_site/_ro/pypackages", "/opt/trn_rl_repo"):
        if os.path.isdir(p) and p not in sys.path:
            sys.path.append(p)
    import concourse.bass as bass

from contextlib import ExitStack

import ml_dtypes
import concourse.tile as tile
from concourse import bacc, bass_utils, mybir
from concourse.masks import make_identity

N_CORES = 8
B = 8192
R = B // N_CORES        # rows per core
D = 1024                # latent dim
KX = 784                # encoder input dim
DT_STEP = 0.2
STEPS = 10
EPS = 1e-5
SW = 16.0               # fp8 weight upscale (exact in bf16/f32)

F32 = mybir.dt.float32
F32R = mybir.dt.float32r
BF16 = mybir.dt.bfloat16
F8 = mybir.dt.float8e4
AF = mybir.ActivationFunctionType
ALU = mybir.AluOpType
DR = mybir.MatmulPerfMode.DoubleRow

KD = D // 128           # 8 k/m tiles over D
NS = R // 512           # 2 moving-dim slices of 512
KXT = [128] * 6 + [16]  # 784 = 6*128 + 16
NWARM = 6               # PE warmup matmuls (HAM un-throttle during DMA wait)


def _build_program():
    nc = bacc.Bacc("TRN2", target_bir_lowering=False, debug=False)

    x = nc.dram_tensor("x", [R, KX], F32, kind="ExternalInput").ap()
    w_c = nc.dram_tensor("w_c", [KX, D], F32, kind="ExternalInput").ap()
    w8 = nc.dram_tensor("w8", [128, KD, D], F8, kind="ExternalInput").ap()
    bias = nc.dram_tensor("bias", [D], F32, kind="ExternalInput").ap()
    w2a = nc.dram_tensor("w2a", [128, KD, 11], BF16, kind="ExternalInput").ap()
    w2r = nc.dram_tensor("w2r", [128, KD, 11], BF16, kind="ExternalInput").ap()
    w1 = nc.dram_tensor("w1", [10], F32, kind="ExternalInput").ap()
    b2 = nc.dram_tensor("b2", [10], F32, kind="ExternalInput").ap()
    out = nc.dram_tensor("out", [R, 10], F32, kind="ExternalOutput").ap()

    with tile.TileContext(nc) as tc, ExitStack() as ctx:
        state = ctx.enter_context(tc.tile_pool(name="state", bufs=1))
        consts = ctx.enter_context(tc.tile_pool(name="consts", bufs=1))
        wres_pool = ctx.enter_context(tc.tile_pool(name="wres", bufs=1))

        # persistent SBUF state (G in fp32, u in bf16, drive holds 16c in f32)
        g = [[state.tile([128, R], F32, name=f"g{b}_{k}", tag=f"g{b}_{k}") for k in range(KD)]
             for b in range(2)]
        u = [state.tile([128, R], BF16, name=f"u{k}", tag=f"u{k}") for k in range(KD)]
        drive = [state.tile([128, R], F32R, name=f"dr{k}", tag=f"dr{k}") for k in range(KD)]
        t8 = state.tile([128, KD, R], F8, name="t8", tag="t8")
        w8_sb = wres_pool.tile([128, KD, D], F8, name="w8", tag="w8")

        with ExitStack() as mmctx:
            psum = mmctx.enter_context(
                tc.tile_pool(name="mm", bufs=4, space="PSUM"))
            if True:
                # PE warmup: dependency-free fp32 matmuls starting at t~0 pull
                # the HAM clock gate to 8/8 while the input DMAs are in flight.
                warm_src = consts.tile([128, 256], F32)
                nc.vector.memset(warm_src, 0.01)
                warm_sb = consts.tile([128, 1], F32)
                for w in range(NWARM):
                    wp = psum.tile([128, 512], F32, name=f"warm{w}", tag="mm")
                    nc.tensor.matmul(wp[:, :256], lhsT=warm_src[:, :128], rhs=warm_src,
                                     start=True, stop=True)
                    if w == NWARM - 1:
                        nc.vector.tensor_copy(warm_sb, wp[:, :1])  # keep-alive

                ident = consts.tile([128, 128], F32)
                make_identity(nc, ident)
                identR = consts.tile([128, 128], F32R)
                nc.vector.tensor_copy(identR, ident)
                bias_sb = consts.tile([128, KD], F32)
                nc.gpsimd.dma_start(out=bias_sb, in_=bias.rearrange("(m p) -> p m", p=128))
                bias16 = consts.tile([128, KD], F32)
                nc.scalar.mul(bias16, bias_sb, SW)

                # ------------ encoder: x -> x.T, 16c = 16(x @ W_c + bias) ----
                with ExitStack() as enc:
                    xn_pool = enc.enter_context(tc.tile_pool(name="xn", bufs=4))
                    xt_pool = enc.enter_context(tc.tile_pool(name="xt", bufs=1))
                    wc_pool = enc.enter_context(tc.tile_pool(name="wc", bufs=1))
                    etp = enc.enter_context(
                        tc.tile_pool(name="etp", bufs=4, space="PSUM"))

                    xt_big = xt_pool.tile([128, len(KXT), R], F32R, name="xt_big")
                    wc_sb = [wc_pool.tile([128, D], F32R, name=f"wc{k}", tag=f"wc{k}")
                             for k in range(len(KXT))]
                    for k, kw in enumerate(KXT):
                        nc.scalar.dma_start(out=wc_sb[k][:kw, :],
                                            in_=w_c[k * 128:k * 128 + kw, :].bitcast(F32R))

                    def transpose_rt(rt):
                        xn = xn_pool.tile([128, KX], F32, name=f"xn{rt}", tag="xn")
                        nc.sync.dma_start(out=xn, in_=x[rt * 128:(rt + 1) * 128, :])
                        rsl = slice(rt * 128, (rt + 1) * 128)
                        wp0 = psum.tile([128, 512], F32, name=f"wmh{rt}", tag="mm")
                        nc.tensor.matmul(wp0[:, :256], lhsT=warm_src[:, :128],
                                         rhs=warm_src, start=True, stop=True)
                        for kp in range(3):
                            pt = etp.tile([128, 256], F32, name=f"pt{rt}_{kp}", tag="tp")
                            for h in range(2):
                                k = 2 * kp + h
                                nc.tensor.transpose(pt[:, h * 128:(h + 1) * 128],
                                                    xn[:, k * 128:(k + 1) * 128], ident)
                            src = pt.rearrange("p (two c) -> p two c", two=2)
                            dst = xt_big[:, 2 * kp:2 * kp + 2, rsl]
                            if kp % 2 == 0:
                                nc.scalar.copy(dst, src)
                            else:
                                nc.vector.tensor_copy(dst, src)
                        pt = etp.tile([128, 256], F32, name=f"pt{rt}_3", tag="tp")
                        nc.tensor.transpose(pt[:16, :128], xn[:, 768:784], ident)
                        nc.vector.tensor_copy(xt_big[:16, 6, rsl], pt[:16, :128])

                    def encoder_mms(n):
                        sl = slice(n * 512, (n + 1) * 512)
                        for m in range(KD):
                            ps = psum.tile([128, 512], F32, name=f"eps{n}_{m}", tag="mm")
                            for k, kw in enumerate(KXT):
                                nc.tensor.matmul(
                                    ps,
                                    lhsT=wc_sb[k][:kw, m * 128:(m + 1) * 128],
                                    rhs=xt_big[:kw, k, sl],
                                    start=(k == 0), stop=(k == len(KXT) - 1))
                            nc.scalar.activation(drive[m][:, sl], ps, AF.Identity,
                                                 bias=bias16[:, m:m + 1], scale=SW)

                    for rt in range(4):
                        transpose_rt(rt)
                    encoder_mms(0)
                    for rt in range(4, 8):
                        transpose_rt(rt)
                    encoder_mms(1)

                # W8 arrives on the gpsimd queue while the encoder runs.
                nc.gpsimd.dma_start(out=w8_sb, in_=w8)

                # ------------ Euler integration loop (16z/0.8^k frame) --------
                tau_pool = ctx.enter_context(tc.tile_pool(name="tau", bufs=4))
                sqp = ctx.enter_context(tc.tile_pool(name="sq", bufs=1))
                sq_tiles = [sqp.tile([128, R], BF16, name=f"sq{k}", tag=f"sq{k}")
                            for k in range(KD)]

                loopctx = ExitStack()
                psum2 = loopctx.enter_context(
                    tc.tile_pool(name="mm2", bufs=2, space="PSUM"))
                for s in range(STEPS):
                    ak = float(0.8 ** s / SW)    # ACT scale: G -> z
                    qk = float(DT_STEP * 1.25 ** (s + 1))  # G-update scalar
                    cur = drive if s == 0 else g[s % 2]
                    nxt = g[(s + 1) % 2]
                    # tanh (bf16) -> fp8 cast (ACT) + u update (DVE)
                    for m in range(KD):
                        tau = tau_pool.tile([128, R], BF16,
                                            name=f"tau{s}_{m}", tag="tau")
                        nc.scalar.activation(tau, cur[m], AF.Tanh, scale=ak)
                        nc.scalar.copy(t8[:, m, :], tau)
                        if s == 0:
                            nc.vector.tensor_copy(u[m], tau)
                        else:
                            nc.vector.scalar_tensor_tensor(
                                u[m], in0=u[m], scalar=1.0 - DT_STEP,
                                in1=tau, op0=ALU.mult, op1=ALU.add)
                        if s == STEPS - 1:
                            nc.vector.tensor_mul(sq_tiles[m], u[m], u[m])
                    if s == STEPS - 1:
                        break
                    # PE: psum[m] = 16c + T8 @ W8 over both row slices; the
                    # DoubleRow stationaries are shared between the slices.
                    for m in range(KD):
                        ps = psum2.tile([128, R], F32, name=f"ps{s}_{m}", tag="mm2")
                        for n in range(NS):
                            nc.tensor.matmul(ps[:, n * 512:(n + 1) * 512],
                                             lhsT=identR,
                                             rhs=drive[m][:, n * 512:(n + 1) * 512],
                                             start=True, stop=False)
                        for j in range(KD // 2):
                            lhsT = w8_sb[:, 2 * j:2 * j + 2, m * 128:(m + 1) * 128]
                            for n in range(NS):
                                nc.tensor.matmul(ps[:, n * 512:(n + 1) * 512], lhsT=lhsT,
                                                 rhs=t8[:, 2 * j:2 * j + 2,
                                                        n * 512:(n + 1) * 512],
                                                 perf_mode=DR,
                                                 start=False, stop=(j == KD // 2 - 1))
                        nc.vector.scalar_tensor_tensor(
                            nxt[m], in0=ps, scalar=qk,
                            in1=cur[m], op0=ALU.mult, op1=ALU.add)

                loopctx.close()
                gfin = u

                # ------------ tail: LN stats + readout (matmul part) ----------
                tail = ctx.enter_context(tc.tile_pool(name="tail", bufs=1))

                ones_sb = tail.tile([128, 1], BF16)
                nc.vector.memset(ones_sb, 1.0)
                eps_sb = tail.tile([128, 1], F32)
                nc.vector.memset(eps_sb, EPS)
                # w2a/w2r = bf16 hi/lo split of [0.2*W2.T | ones]
                w2a_sb = tail.tile([128, KD, 11], BF16)
                nc.gpsimd.dma_start(out=w2a_sb, in_=w2a)
                w2r_sb = tail.tile([128, KD, 11], BF16)
                nc.gpsimd.dma_start(out=w2r_sb, in_=w2r)
                w1_bc = tail.tile([128, 10], F32)
                nc.gpsimd.dma_start(out=w1_bc, in_=bass.AP(tensor=w1.tensor, offset=w1.offset,
                                                           ap=[[0, 128]] + list(w1.ap)))
                b2_bc = tail.tile([128, 10], F32)
                nc.gpsimd.dma_start(out=b2_bc, in_=bass.AP(tensor=b2.tensor, offset=b2.offset,
                                                           ap=[[0, 128]] + list(b2.ap)))

                s2_sb = tail.tile([1, R], F32)
                y_sb = tail.tile([11, R], F32)

                tp2ctx = ExitStack()
                tp2 = tp2ctx.enter_context(
                    tc.tile_pool(name="tp2", bufs=4, space="PSUM"))
                for n in range(NS):
                    sl = slice(n * 512, (n + 1) * 512)
                    yp = psum.tile([11, 512], F32, name=f"yp{n}", tag="mm")
                    for k in range(KD):
                        nc.tensor.matmul(yp, lhsT=w2a_sb[:, k, :],
                                         rhs=gfin[k][:, sl],
                                         start=(k == 0), stop=False)
                    for k in range(KD):
                        nc.tensor.matmul(yp, lhsT=w2r_sb[:, k, :],
                                         rhs=gfin[k][:, sl],
                                         start=False, stop=(k == KD - 1))
                    nc.scalar.copy(y_sb[:, sl], yp)
                    s2 = psum.tile([1, 512], F32, name=f"s2p{n}", tag="mm")
                    for k in range(KD):
                        nc.tensor.matmul(s2, lhsT=ones_sb, rhs=sq_tiles[k][:, sl],
                                         start=(k == 0), stop=(k == KD - 1))
                    nc.scalar.copy(s2_sb[:, sl], s2)

                    for rt in range(n * 4, (n + 1) * 4):
                        sl = slice(rt * 128, (rt + 1) * 128)
                        yn = tp2.tile([128, 11], F32, name=f"yn{rt}", tag="st")
                        nc.tensor.transpose(yn, y_sb[:, sl], ident[:11, :11])
                        p2 = tp2.tile([128, 1], F32, name=f"p2_{rt}", tag="st")
                        nc.tensor.transpose(p2, s2_sb[:, sl], ident[:1, :1])
                        mu_n = tail.tile([128, 1], F32, name=f"mu{rt}", tag="mu", bufs=2)
                        nc.scalar.mul(mu_n, yn[:, 10:11], -DT_STEP / D)   # -mean(h)
                        ex2 = tail.tile([128, 1], F32, name=f"ex2_{rt}", tag="ex2", bufs=2)
                        nc.scalar.mul(ex2, p2, DT_STEP * DT_STEP / D)     # E[h^2]
                        var = tail.tile([128, 1], F32, name=f"var{rt}", tag="var", bufs=2)
                        nc.vector.scalar_tensor_tensor(var, in0=mu_n, scalar=-1.0,
                                                       op0=ALU.mult, in1=mu_n,
                                                       op1=ALU.mult)
                        nc.vector.tensor_add(var, var, ex2)
                        sd = tail.tile([128, 1], F32, name=f"sd{rt}", tag="sd", bufs=2)
                        nc.scalar.activation(sd, var, AF.Sqrt, bias=eps_sb, scale=1.0)
                        inv = tail.tile([128, 1], F32, name=f"inv{rt}", tag="inv", bufs=2)
                        nc.vector.reciprocal(inv, sd)
                        qn = tail.tile([128, 1], F32, name=f"qn{rt}", tag="qn", bufs=2)
                        nc.vector.tensor_mul(qn, mu_n, inv)               # -mu*inv

                        t1 = tail.tile([128, 10], F32, name=f"t1_{rt}", tag="t1", bufs=2)
                        nc.vector.tensor_scalar_mul(t1, yn[:, 0:10], inv)
                        t2 = tail.tile([128, 10], F32, name=f"t2_{rt}", tag="t2", bufs=2)
                        nc.vector.scalar_tensor_tensor(t2, in0=w1_bc, scalar=qn,
                                                       in1=t1, op0=ALU.mult, op1=ALU.add)
                        o = tail.tile([128, 10], F32, name=f"o{rt}", tag="o", bufs=2)
                        nc.vector.tensor_add(o, t2, b2_bc)
                        nc.sync.dma_start(out=out[sl, :], in_=o)
                tp2ctx.close()

    nc.compile()
    return nc


_NC_CACHE = None


def _get_program():
    global _NC_CACHE
    if _NC_CACHE is None:
        _NC_CACHE = _build_program()
    return _NC_CACHE


def _prepare_in_maps(inputs):
    x = np.asarray(inputs["x"], dtype=np.float32)
    w_enc = np.asarray(inputs["W_enc"], dtype=np.float32)
    w_res = np.asarray(inputs["W_res"], dtype=np.float32)
    w_in = np.asarray(inputs["W_in"], dtype=np.float32)
    bias = np.asarray(inputs["bias"], dtype=np.float32)
    ln_g = np.asarray(inputs["ln_g"], dtype=np.float32)
    ln_b = np.asarray(inputs["ln_b"], dtype=np.float32)
    w_out = np.asarray(inputs["W_out"], dtype=np.float32)
    b_out = np.asarray(inputs["b_out"], dtype=np.float32)

    w_c = (w_enc.T.astype(np.float64) @ w_in.astype(np.float64)).astype(np.float32)
    w2 = w_out * ln_g[None, :]                       # [10, D]

    # fp8 recurrent weights, upscaled by SW, layout [p, ksub, m]
    w8 = (SW * w_res).astype(ml_dtypes.float8_e4m3)
    w8 = np.ascontiguousarray(w8.reshape(KD, 128, D).transpose(1, 0, 2))

    # readout: [0.2*W2.T | ones] in bf16 hi + bf16 residual, layout [p, k, o]
    a = np.empty((D, 11), np.float64)
    a[:, :10] = DT_STEP * w2.T.astype(np.float64)
    a[:, 10] = 1.0
    a16 = a.astype(ml_dtypes.bfloat16)
    ar16 = (a - a16.astype(np.float64)).astype(ml_dtypes.bfloat16)
    a16 = np.ascontiguousarray(a16.reshape(KD, 128, 11).transpose(1, 0, 2))
    ar16 = np.ascontiguousarray(ar16.reshape(KD, 128, 11).transpose(1, 0, 2))

    w1v = w2.sum(axis=1).astype(np.float32)
    b2v = (w_out.astype(np.float64) @ ln_b.astype(np.float64)
           + b_out.astype(np.float64)).astype(np.float32)

    shared = {
        "w_c": np.ascontiguousarray(w_c),
        "w8": w8,
        "bias": np.ascontiguousarray(bias),
        "w2a": a16,
        "w2r": ar16,
        "w1": np.ascontiguousarray(w1v),
        "b2": np.ascontiguousarray(b2v),
    }
    in_maps = []
    for c in range(N_CORES):
        m = dict(shared)
        m["x"] = np.ascontiguousarray(x[c * R:(c + 1) * R, :])
        in_maps.append(m)
    return in_maps


def run(inputs, trace=False, tmpdir=None):
    """Run on 8 NeuronCores; returns (out [8192,10], BassKernelResults)."""
    nc = _get_program()
    in_maps = _prepare_in_maps(inputs)
    res = bass_utils.run_bass_kernel_spmd(
        nc, in_maps, core_ids=list(range(N_CORES)), trace=trace, tmpdir=tmpdir)
    outs = [np.asarray(r["out"]) for r in res.results]
    return np.concatenate(outs, axis=0), res


def kernel(**inputs):
    out, _ = run(inputs, trace=False)
    return out


# revision 17
# speedup vs baseline: 1.4888x; 1.0092x over previous
"""Trainium2 Bass kernel for nn_ChimeraNet (encoder -> 10-step Euler RNN -> LN -> readout).

Data-parallel over 8 NeuronCores: each core gets 1024 rows of the batch and a
replicated set of (host-prefolded) weights.

Math (per core, R=1024 rows, D=1024), in "drive space" z = h @ W_res + c:
    c   = x @ W_c + bias               with W_c = W_enc.T @ W_in (host-folded)
    z_0 = c;  T_k = tanh(z_k)
    z_{k+1} = 0.8 z_k + 0.2 c + 0.2 (T_k @ W_res)      k = 0..8
    u_{k+1} = 0.8 u_k + T_k                            k = 0..9,  u_0 = 0
    h = 0.2 u_10;  out = LayerNorm(h) @ W_out.T + b_out (folded like before)

The z state is kept in the exponentially rescaled+upscaled frame
G_k = 16 z_k / 0.8^k (fp32) so each step's state update is a single
one-scalar DVE op reading the matmul PSUM directly:
    G_{k+1} = G_k + 1.25^{k+1} * psum
    psum    = 16 c (f32r identity matmul) + T8 @ fp8(16 W_res)  (DoubleRow fp8)
    T_k     = tanh((0.8^k/16) * G_k)   (ACT with scale, bf16 out)
The drive tiles store 16c (scale folded into the encoder eviction), so G_0 IS
the drive tile and no init op is needed.

fp8 e4m3 is used ONLY for the recurrent matmul operands (T8 = fp8 cast of the
bf16 tanh, on ACT; W8 = fp8(16*W_res)); the u accumulator consumes the exact
bf16 tanh, which keeps the final relative error ~7e-3.
DoubleRow contracts 2 k-subtiles per matmul at 0.5 cyc/row -> ~2x PE on the
dominant recurrent matmul.

Elementwise ops run at FD=1024 (full row range) to amortize per-op overheads;
the per-m-tile PSUM is a 2-bank [128,1024] tile whose halves are filled by
N=512 matmuls (both row slices share each DoubleRow stationary back-to-back).
"""

import os
import sys

import numpy as np

try:
    import concourse.bass as bass  # noqa: F401
except ImportError:  # pragma: no cover - fresh grading env without PYTHONPATH
    for p in ("/root/.axon_site", "/root/.axon_site/_ro/trn_rl_repo",
              "/root/.axon_site/_ro/pypackages", "/opt/trn_rl_repo"):
        if os.path.isdir(p) and p not in sys.path:
            sys.path.append(p)
    import concourse.bass as bass

from contextlib import ExitStack

import ml_dtypes
import concourse.tile as tile
from concourse import bacc, bass_utils, mybir
from concourse.masks import make_identity

N_CORES = 8
B = 8192
R = B // N_CORES        # rows per core
D = 1024                # latent dim
KX = 784                # encoder input dim
DT_STEP = 0.2
STEPS = 10
EPS = 1e-5
SW = 16.0               # fp8 weight upscale (exact in bf16/f32)

F32 = mybir.dt.float32
F32R = mybir.dt.float32r
BF16 = mybir.dt.bfloat16
F8 = mybir.dt.float8e4
AF = mybir.ActivationFunctionType
ALU = mybir.AluOpType
DR = mybir.MatmulPerfMode.DoubleRow

KD = D // 128           # 8 k/m tiles over D
NS = R // 512           # 2 moving-dim slices of 512
KXT = [128] * 6 + [16]  # 784 = 6*128 + 16
NWARM = 6               # PE warmup matmuls (HAM un-throttle during DMA wait)


def _build_program():
    nc = bacc.Bacc("TRN2", target_bir_lowering=False, debug=False)

    x = nc.dram_tensor("x", [R, KX], F32, kind="ExternalInput").ap()
    w_c = nc.dram_tensor("w_c", [KX, D], F32, kind="ExternalInput").ap()
    w8 = nc.dram_tensor("w8", [128, KD, D], F8, kind="ExternalInput").ap()
    bias = nc.dram_tensor("bias", [D], F32, kind="ExternalInput").ap()
    w2a = nc.dram_tensor("w2a", [128, KD, 11], BF16, kind="ExternalInput").ap()
    w2r = nc.dram_tensor("w2r", [128, KD, 11], BF16, kind="ExternalInput").ap()
    w1 = nc.dram_tensor("w1", [10], F32, kind="ExternalInput").ap()
    b2 = nc.dram_tensor("b2", [10], F32, kind="ExternalInput").ap()
    out = nc.dram_tensor("out", [R, 10], F32, kind="ExternalOutput").ap()

    with tile.TileContext(nc) as tc, ExitStack() as ctx:
        state = ctx.enter_context(tc.tile_pool(name="state", bufs=1))
        consts = ctx.enter_context(tc.tile_pool(name="consts", bufs=1))
        wres_pool = ctx.enter_context(tc.tile_pool(name="wres", bufs=1))

        # persistent SBUF state (G in fp32 updated in place, u in bf16,
        # drive holds 16c in f32r)
        g = [state.tile([128, R], F32, name=f"g{k}", tag=f"g{k}") for k in range(KD)]
        u = [state.tile([128, R], BF16, name=f"u{k}", tag=f"u{k}") for k in range(KD)]
        drive = [state.tile([128, R], F32R, name=f"dr{k}", tag=f"dr{k}") for k in range(KD)]
        t8 = state.tile([128, KD, R], F8, name="t8", tag="t8")
        w8_sb = wres_pool.tile([128, KD, D], F8, name="w8", tag="w8")

        with ExitStack() as mmctx:
            psum = mmctx.enter_context(
                tc.tile_pool(name="mm", bufs=4, space="PSUM"))
            if True:
                # PE warmup: dependency-free fp32 matmuls starting at t~0 pull
                # the HAM clock gate to 8/8 while the input DMAs are in flight.
                warm_src = consts.tile([128, 256], F32)
                nc.vector.memset(warm_src, 0.01)
                warm_sb = consts.tile([128, 1], F32)
                for w in range(NWARM):
                    wp = psum.tile([128, 512], F32, name=f"warm{w}", tag="mm")
                    nc.tensor.matmul(wp[:, :256], lhsT=warm_src[:, :128], rhs=warm_src,
                                     start=True, stop=True)
                    if w == NWARM - 1:
                        nc.vector.tensor_copy(warm_sb, wp[:, :1])  # keep-alive

                ident = consts.tile([128, 128], F32)
                make_identity(nc, ident)
                identR = consts.tile([128, 128], F32R)
                nc.vector.tensor_copy(identR, ident)
                bias_sb = consts.tile([128, KD], F32)
                nc.gpsimd.dma_start(out=bias_sb, in_=bias.rearrange("(m p) -> p m", p=128))
                bias16 = consts.tile([128, KD], F32)
                nc.scalar.mul(bias16, bias_sb, SW)

                # loop pools created up front so the step-0 prologue can be
                # interleaved with the encoder's slice-1 evictions.
                tau_pool = ctx.enter_context(tc.tile_pool(name="tau", bufs=4))

                def prologue_m(m):
                    # T_0 = tanh(z_0) from the drive tile (G_0 = 16c)
                    tau = tau_pool.tile([128, R], BF16, name=f"tau0_{m}", tag="tau")
                    nc.scalar.activation(tau, drive[m], AF.Tanh, scale=float(1.0 / SW))
                    nc.scalar.copy(t8[:, m, :], tau)
                    nc.vector.tensor_copy(u[m], tau)

                # ------------ encoder: x -> x.T, 16c = 16(x @ W_c + bias) ----
                with ExitStack() as enc:
                    xn_pool = enc.enter_context(tc.tile_pool(name="xn", bufs=4))
                    xt_pool = enc.enter_context(tc.tile_pool(name="xt", bufs=1))
                    wc_pool = enc.enter_context(tc.tile_pool(name="wc", bufs=1))
                    etp = enc.enter_context(
                        tc.tile_pool(name="etp", bufs=4, space="PSUM"))

                    xt_big = xt_pool.tile([128, len(KXT), R], F32R, name="xt_big")
                    wc_sb = [wc_pool.tile([128, D], F32R, name=f"wc{k}", tag=f"wc{k}")
                             for k in range(len(KXT))]
                    for k, kw in enumerate(KXT):
                        nc.scalar.dma_start(out=wc_sb[k][:kw, :],
                                            in_=w_c[k * 128:k * 128 + kw, :].bitcast(F32R))

                    def transpose_rt(rt):
                        xn = xn_pool.tile([128, KX], F32, name=f"xn{rt}", tag="xn")
                        nc.sync.dma_start(out=xn, in_=x[rt * 128:(rt + 1) * 128, :])
                        rsl = slice(rt * 128, (rt + 1) * 128)
                        wp0 = psum.tile([128, 512], F32, name=f"wmh{rt}", tag="mm")
                        nc.tensor.matmul(wp0[:, :256], lhsT=warm_src[:, :128],
                                         rhs=warm_src, start=True, stop=True)
                        for kp in range(3):
                            pt = etp.tile([128, 256], F32, name=f"pt{rt}_{kp}", tag="tp")
                            for h in range(2):
                                k = 2 * kp + h
                                nc.tensor.transpose(pt[:, h * 128:(h + 1) * 128],
                                                    xn[:, k * 128:(k + 1) * 128], ident)
                            src = pt.rearrange("p (two c) -> p two c", two=2)
                            dst = xt_big[:, 2 * kp:2 * kp + 2, rsl]
                            if kp % 2 == 0:
                                nc.scalar.copy(dst, src)
                            else:
                                nc.vector.tensor_copy(dst, src)
                        pt = etp.tile([128, 256], F32, name=f"pt{rt}_3", tag="tp")
                        nc.tensor.transpose(pt[:16, :128], xn[:, 768:784], ident)
                        nc.vector.tensor_copy(xt_big[:16, 6, rsl], pt[:16, :128])

                    def encoder_mms(n, post_evict=None):
                        sl = slice(n * 512, (n + 1) * 512)
                        for m in range(KD):
                            ps = psum.tile([128, 512], F32, name=f"eps{n}_{m}", tag="mm")
                            for k, kw in enumerate(KXT):
                                nc.tensor.matmul(
                                    ps,
                                    lhsT=wc_sb[k][:kw, m * 128:(m + 1) * 128],
                                    rhs=xt_big[:kw, k, sl],
                                    start=(k == 0), stop=(k == len(KXT) - 1))
                            nc.scalar.activation(drive[m][:, sl], ps, AF.Identity,
                                                 bias=bias16[:, m:m + 1], scale=SW)
                            if post_evict is not None:
                                post_evict(m)

                    for rt in range(4):
                        transpose_rt(rt)
                    encoder_mms(0)
                    for rt in range(4, 8):
                        transpose_rt(rt)
                    encoder_mms(1, post_evict=prologue_m)

                # W8 arrives on the gpsimd queue while the encoder runs.
                nc.gpsimd.dma_start(out=w8_sb, in_=w8)

                sqp = ctx.enter_context(tc.tile_pool(name="sq", bufs=1))
                sq_tiles = [sqp.tile([128, R], BF16, name=f"sq{k}", tag=f"sq{k}")
                            for k in range(KD)]

                # ------------ Euler integration loop (16z/0.8^k frame) --------
                # Software-pipelined issue order per step s (s = 0..8):
                #   PE:  8 matmul groups (16c identity + fp8-DR, both slices)
                #   DVE: 8 G-updates (read psum)    -- ahead of u's in the FIFO
                #   ACT: tanh(s+1) + fp8 cast(s+1)  -- overlaps next step's PE
                #   DVE: u-updates for step s+1 (+ squares at the last step)
                loopctx = ExitStack()
                psum2 = loopctx.enter_context(
                    tc.tile_pool(name="mm2", bufs=2, space="PSUM"))
                for s in range(STEPS - 1):
                    ak1 = float(0.8 ** (s + 1) / SW)       # tanh scale, step s+1
                    qk = float(DT_STEP * 1.25 ** (s + 1))  # G-update scalar
                    cur = drive if s == 0 else g
                    nxt = g
                    last = (s + 1 == STEPS - 1)
                    pss = []
                    for m in range(KD):
                        ps = psum2.tile([128, R], F32, name=f"ps{s}_{m}", tag="mm2")
                        for n in range(NS):
                            nc.tensor.matmul(ps[:, n * 512:(n + 1) * 512],
                                             lhsT=identR,
                                             rhs=drive[m][:, n * 512:(n + 1) * 512],
                                             start=True, stop=False)
                        for j in range(KD // 2):
                            lhsT = w8_sb[:, 2 * j:2 * j + 2, m * 128:(m + 1) * 128]
                            for n in range(NS):
                                nc.tensor.matmul(ps[:, n * 512:(n + 1) * 512], lhsT=lhsT,
                                                 rhs=t8[:, 2 * j:2 * j + 2,
                                                        n * 512:(n + 1) * 512],
                                                 perf_mode=DR,
                                                 start=False, stop=(j == KD // 2 - 1))
                        pss.append(ps)
                    for m in range(KD):
                        nc.vector.scalar_tensor_tensor(
                            nxt[m], in0=pss[m], scalar=qk,
                            in1=cur[m], op0=ALU.mult, op1=ALU.add)
                    for m in range(KD):
                        tau = tau_pool.tile([128, R], BF16,
                                            name=f"tau{s + 1}_{m}", tag="tau")
                        nc.scalar.activation(tau, nxt[m], AF.Tanh, scale=ak1)
                        if not last:
                            nc.scalar.copy(t8[:, m, :], tau)
                        nc.vector.scalar_tensor_tensor(
                            u[m], in0=u[m], scalar=1.0 - DT_STEP,
                            in1=tau, op0=ALU.mult, op1=ALU.add)
                        if last:
                            nc.vector.tensor_mul(sq_tiles[m], u[m], u[m])

                loopctx.close()
                gfin = u

                # ------------ tail: LN stats + readout (matmul part) ----------
                tail = ctx.enter_context(tc.tile_pool(name="tail", bufs=1))

                ones_sb = tail.tile([128, 1], BF16)
                nc.vector.memset(ones_sb, 1.0)
                eps_sb = tail.tile([128, 1], F32)
                nc.vector.memset(eps_sb, EPS)
                # w2a/w2r = bf16 hi/lo split of [0.2*W2.T | ones]
                w2a_sb = tail.tile([128, KD, 11], BF16)
                nc.gpsimd.dma_start(out=w2a_sb, in_=w2a)
                w2r_sb = tail.tile([128, KD, 11], BF16)
                nc.gpsimd.dma_start(out=w2r_sb, in_=w2r)
                w1_bc = tail.tile([128, 10], F32)
                nc.gpsimd.dma_start(out=w1_bc, in_=bass.AP(tensor=w1.tensor, offset=w1.offset,
                                                           ap=[[0, 128]] + list(w1.ap)))
                b2_bc = tail.tile([128, 10], F32)
                nc.gpsimd.dma_start(out=b2_bc, in_=bass.AP(tensor=b2.tensor, offset=b2.offset,
                                                           ap=[[0, 128]] + list(b2.ap)))

                s2_sb = tail.tile([1, R], F32)
                y_sb = tail.tile([11, R], F32)

                tp2ctx = ExitStack()
                tp2 = tp2ctx.enter_context(
                    tc.tile_pool(name="tp2", bufs=4, space="PSUM"))
                for n in range(NS):
                    sl = slice(n * 512, (n + 1) * 512)
                    yp = psum.tile([11, 512], F32, name=f"yp{n}", tag="mm")
                    for k in range(KD):
                        nc.tensor.matmul(yp, lhsT=w2a_sb[:, k, :],
                                         rhs=gfin[k][:, sl],
                                         start=(k == 0), stop=False)
                    for k in range(KD):
                        nc.tensor.matmul(yp, lhsT=w2r_sb[:, k, :],
                                         rhs=gfin[k][:, sl],
                                         start=False, stop=(k == KD - 1))
                    nc.scalar.copy(y_sb[:, sl], yp)
                    s2 = psum.tile([1, 512], F32, name=f"s2p{n}", tag="mm")
                    for k in range(KD):
                        nc.tensor.matmul(s2, lhsT=ones_sb, rhs=sq_tiles[k][:, sl],
                                         start=(k == 0), stop=(k == KD - 1))
                    nc.scalar.copy(s2_sb[:, sl], s2)

                    for rt in range(n * 4, (n + 1) * 4):
                        sl = slice(rt * 128, (rt + 1) * 128)
                        yn = tp2.tile([128, 11], F32, name=f"yn{rt}", tag="st")
                        nc.tensor.transpose(yn, y_sb[:, sl], ident[:11, :11])
                        p2 = tp2.tile([128, 1], F32, name=f"p2_{rt}", tag="st")
                        nc.tensor.transpose(p2, s2_sb[:, sl], ident[:1, :1])
                        mu_n = tail.tile([128, 1], F32, name=f"mu{rt}", tag="mu", bufs=2)
                        nc.scalar.mul(mu_n, yn[:, 10:11], -DT_STEP / D)   # -mean(h)
                        ex2 = tail.tile([128, 1], F32, name=f"ex2_{rt}", tag="ex2", bufs=2)
                        nc.scalar.mul(ex2, p2, DT_STEP * DT_STEP / D)     # E[h^2]
                        var = tail.tile([128, 1], F32, name=f"var{rt}", tag="var", bufs=2)
                        nc.vector.scalar_tensor_tensor(var, in0=mu_n, scalar=-1.0,
                                                       op0=ALU.mult, in1=mu_n,
                                                       op1=ALU.mult)
                        nc.vector.tensor_add(var, var, ex2)
                        sd = tail.tile([128, 1], F32, name=f"sd{rt}", tag="sd", bufs=2)
                        nc.scalar.activation(sd, var, AF.Sqrt, bias=eps_sb, scale=1.0)
                        inv = tail.tile([128, 1], F32, name=f"inv{rt}", tag="inv", bufs=2)
                        nc.vector.reciprocal(inv, sd)
                        qn = tail.tile([128, 1], F32, name=f"qn{rt}", tag="qn", bufs=2)
                        nc.vector.tensor_mul(qn, mu_n, inv)               # -mu*inv

                        t1 = tail.tile([128, 10], F32, name=f"t1_{rt}", tag="t1", bufs=2)
                        nc.vector.tensor_scalar_mul(t1, yn[:, 0:10], inv)
                        t2 = tail.tile([128, 10], F32, name=f"t2_{rt}", tag="t2", bufs=2)
                        nc.vector.scalar_tensor_tensor(t2, in0=w1_bc, scalar=qn,
                                                       in1=t1, op0=ALU.mult, op1=ALU.add)
                        o = tail.tile([128, 10], F32, name=f"o{rt}", tag="o", bufs=2)
                        nc.vector.tensor_add(o, t2, b2_bc)
                        nc.sync.dma_start(out=out[sl, :], in_=o)
                tp2ctx.close()

    nc.compile()
    return nc


_NC_CACHE = None


def _get_program():
    global _NC_CACHE
    if _NC_CACHE is None:
        _NC_CACHE = _build_program()
    return _NC_CACHE


def _prepare_in_maps(inputs):
    x = np.asarray(inputs["x"], dtype=np.float32)
    w_enc = np.asarray(inputs["W_enc"], dtype=np.float32)
    w_res = np.asarray(inputs["W_res"], dtype=np.float32)
    w_in = np.asarray(inputs["W_in"], dtype=np.float32)
    bias = np.asarray(inputs["bias"], dtype=np.float32)
    ln_g = np.asarray(inputs["ln_g"], dtype=np.float32)
    ln_b = np.asarray(inputs["ln_b"], dtype=np.float32)
    w_out = np.asarray(inputs["W_out"], dtype=np.float32)
    b_out = np.asarray(inputs["b_out"], dtype=np.float32)

    w_c = (w_enc.T.astype(np.float64) @ w_in.astype(np.float64)).astype(np.float32)
    w2 = w_out * ln_g[None, :]                       # [10, D]

    # fp8 recurrent weights, upscaled by SW, layout [p, ksub, m]
    w8 = (SW * w_res).astype(ml_dtypes.float8_e4m3)
    w8 = np.ascontiguousarray(w8.reshape(KD, 128, D).transpose(1, 0, 2))

    # readout: [0.2*W2.T | ones] in bf16 hi + bf16 residual, layout [p, k, o]
    a = np.empty((D, 11), np.float64)
    a[:, :10] = DT_STEP * w2.T.astype(np.float64)
    a[:, 10] = 1.0
    a16 = a.astype(ml_dtypes.bfloat16)
    ar16 = (a - a16.astype(np.float64)).astype(ml_dtypes.bfloat16)
    a16 = np.ascontiguousarray(a16.reshape(KD, 128, 11).transpose(1, 0, 2))
    ar16 = np.ascontiguousarray(ar16.reshape(KD, 128, 11).transpose(1, 0, 2))

    w1v = w2.sum(axis=1).astype(np.float32)
    b2v = (w_out.astype(np.float64) @ ln_b.astype(np.float64)
           + b_out.astype(np.float64)).astype(np.float32)

    shared = {
        "w_c": np.ascontiguousarray(w_c),
        "w8": w8,
        "bias": np.ascontiguousarray(bias),
        "w2a": a16,
        "w2r": ar16,
        "w1": np.ascontiguousarray(w1v),
        "b2": np.ascontiguousarray(b2v),
    }
    in_maps = []
    for c in range(N_CORES):
        m = dict(shared)
        m["x"] = np.ascontiguousarray(x[c * R:(c + 1) * R, :])
        in_maps.append(m)
    return in_maps


def run(inputs, trace=False, tmpdir=None):
    """Run on 8 NeuronCores; returns (out [8192,10], BassKernelResults)."""
    nc = _get_program()
    in_maps = _prepare_in_maps(inputs)
    res = bass_utils.run_bass_kernel_spmd(
        nc, in_maps, core_ids=list(range(N_CORES)), trace=trace, tmpdir=tmpdir)
    outs = [np.asarray(r["out"]) for r in res.results]
    return np.concatenate(outs, axis=0), res


def kernel(**inputs):
    out, _ = run(inputs, trace=False)
    return out
